# revision 11
# baseline (speedup 1.0000x reference)
"""Trainium2 Bass kernel for nn_DSRLossStateless (DSR loss, stateless).

loss = -sum_t(D_t)/B where D_t comes from an eta-EMA pair (A,B) over
portfolio returns R_t = sum_a w[t,a]*nr[t,a].

v4 strategy (8 cores, batch-sharded, bf16 inputs):
  - Host casts w/nr to bf16 (tolerance 2e-2; quantization costs ~2e-3).
    Each core owns 250k consecutive rows as 126 partitions x 2000 steps;
    partition 0 holds the 2000 preceding rows (synthetic for core 0)
    whose local scan final supplies the carry for partition 1
    (c^2000 ~ 1.9e-9 kills older terms).
  - The loss is dominated by the t=0 cold start (q_0 ~ -R_0/sqrt(eps)),
    so the synthetic prepend must reproduce (A,B)=(~0,~eps) to ~1e-4:
    the last two rows encode returns (R_x, -c*R_x) with c*R_x spread
    over three bf16 assets (cancellation residual ~1e-10) and a trim
    asset u tuned on host by exact f32 simulation of the device
    pipeline so B lands on eps.
  - 4 chunks of 500 steps on three DMA queues (Sync/Scalar HWDGE get
    partitions 0:84 of w/nr, gpsimd SWDGE gets 84:126 of both). DVE
    multiplies in bf16 (2x mode) and reduces by 16 via a bf16 fold
    tree (8+8 -> 4 -> 2 -> 1, last two folds f32); ACT derives eta*R,
    eta*R^2, R^2 per chunk.
  - Tail: two full-length scans, SWDGE carry gathers, fused correction
    A_prev = Aloc + carry*c^t, D-chain split across DVE/ACT/gpsimd with
    the eps-clamp fused into the var^1.5 multiply, fused accumulate,
    SWDGE partition gather + f32 DVE reduce (no PE anywhere - PE fp32
    is reduced precision and the partials cancel heavily).
  - Host: loss = eta * sum(partials) / B.
"""

import sys

sys.path.insert(0, "/opt/trn_rl_repo")

import numpy as np
import ml_dtypes

import concourse.bass as bass
import concourse.bacc as bacc
import concourse.tile as tile
from concourse import mybir
from concourse.bass_utils import run_bass_kernel_spmd
from contextlib import ExitStack

F32 = mybir.dt.float32
BF16 = mybir.dt.bfloat16
NF32 = np.float32
NBF16 = ml_dtypes.bfloat16

N_CORES = 8
NA = 16            # assets
KP = 126           # SBUF partitions (0 = prepend/carry-feeder)
L = 2000           # steps per partition
LE = L + 1         # scan buffer width (col 0 = zero carry)
OWN = (KP - 1) * L      # rows owned per core = 250000
B_TOTAL = N_CORES * OWN # 2000000
CHUNKS = [(0, 250), (250, 450), (700, 450), (1150, 450), (1600, 400)]
KC = 450           # max chunk width (tile sizing)
FW = KC * NA       # 8000 bf16 per partition per chunk tile
HQ = 84            # partitions 0:84 -> HWDGE queues, 84:126 -> SWDGE
ETA = 0.01
EPS = 1e-8
CDEC = NF32(1.0 - ETA)  # 0.99

AL = mybir.AluOpType
AF = mybir.ActivationFunctionType
AX = mybir.AxisListType

_PROGRAM = None


def _build_program():
    nc = bacc.Bacc("TRN2", target_bir_lowering=False, debug=False)

    w_ap = nc.dram_tensor("w", [KP * L, NA], BF16, kind="ExternalInput").ap()
    nr_ap = nc.dram_tensor("nr", [KP * L, NA], BF16, kind="ExternalInput").ap()
    out_ap = nc.dram_tensor("out", [1, 1], F32, kind="ExternalOutput").ap()

    # geo_c[t] = c^t (carry decay for the correction pass)
    geoc_np = (CDEC ** np.arange(L).astype(NF32)).astype(NF32)
    geoc_dram = nc.inline_tensor(
        np.ascontiguousarray(np.broadcast_to(geoc_np, (KP, L))), name="geoc"
    )

    w_v = w_ap.rearrange("(p t) a -> p (t a)", p=KP)
    nr_v = nr_ap.rearrange("(p t) a -> p (t a)", p=KP)

    with tile.TileContext(nc) as tc, ExitStack() as ctx:
        pers = ctx.enter_context(tc.tile_pool(name="pers", bufs=1))
        loadp = ctx.enter_context(tc.tile_pool(name="load", bufs=3))
        prodp = ctx.enter_context(tc.tile_pool(name="prod", bufs=1))

        R = pers.tile([KP, L], F32, tag="R")
        R2 = pers.tile([KP, L], F32, tag="R2")
        etaR = pers.tile([KP, L], F32, tag="etaR")
        etaR2 = pers.tile([KP, L], F32, tag="etaR2")
        Aloc = pers.tile([KP, LE], F32, tag="Aloc")
        Bloc = pers.tile([KP, LE], F32, tag="Bloc")
        Aprev = pers.tile([KP, L], F32, tag="Aprev")
        Bprev = pers.tile([KP, L], F32, tag="Bprev")
        geoc = pers.tile([KP, L], F32, tag="geoc")
        cvecL = pers.tile([KP, KC], F32, tag="cvecL")
        T1 = pers.tile([KP, L], F32, tag="T1")
        initA = pers.tile([KP, 1], F32, tag="initA")
        initB = pers.tile([KP, 1], F32, tag="initB")
        qsum = pers.tile([KP, 1], F32, tag="qsum")
        qrow = pers.tile([1, KP - 1], F32, tag="qrow")
        qtot = pers.tile([1, 1], F32, tag="qtot")

        # constants / scan seeds / ACT table pin
        nc.vector.memset(qtot[0:1, 0:1], 1.0)
        nc.scalar.sqrt(qtot[0:1, 0:1], qtot[0:1, 0:1])
        nc.vector.memset(cvecL[:, :], float(CDEC))  # scan decay const
        nc.vector.memset(Aloc[:, 0:1], 0.0)
        nc.vector.memset(Bloc[:, 0:1], 0.0)
        nc.vector.memset(initA[0:1, 0:1], 0.0)
        nc.vector.memset(initB[0:1, 0:1], 0.0)

        # ---- stage A: chunked bulk (small first chunk shortens the DMA
        # ramp; small last chunk starts the tail sooner) ----
        for k0, kc in CHUNKS:
            ks = slice(k0, k0 + kc)
            cs = slice(k0 * NA, (k0 + kc) * NA)
            fw = kc * NA
            wt = loadp.tile([KP, FW], BF16, tag="wt")
            rt = loadp.tile([KP, FW], BF16, tag="rt")
            nc.sync.dma_start(wt[:, 0:fw], w_v[:, cs])
            nc.scalar.dma_start(rt[:, 0:fw], nr_v[:, cs])
            prod = prodp.tile([KP, FW], BF16, tag="prod")
            f8 = prodp.tile([KP, KC * 8], BF16, tag="f8")
            f4 = prodp.tile([KP, KC * 4], BF16, tag="f4")
            f2 = prodp.tile([KP, KC * 2], BF16, tag="f2")
            nc.vector.tensor_mul(prod[:, 0:fw], wt[:, 0:fw], rt[:, 0:fw])
            pv = prod[:, 0:fw].rearrange("p (t a) -> p t a", a=16)
            nc.vector.tensor_add(
                f8[:, 0:kc * 8].rearrange("p (t a) -> p t a", a=8),
                pv[:, :, 0:8], pv[:, :, 8:16],
            )
            v8 = f8[:, 0:kc * 8].rearrange("p (t a) -> p t a", a=8)
            nc.vector.tensor_add(
                f4[:, 0:kc * 4].rearrange("p (t a) -> p t a", a=4),
                v8[:, :, 0:4], v8[:, :, 4:8],
            )
            v4 = f4[:, 0:kc * 4].rearrange("p (t a) -> p t a", a=4)
            nc.vector.tensor_add(
                f2[:, 0:kc * 2].rearrange("p (t a) -> p t a", a=2),
                v4[:, :, 0:2], v4[:, :, 2:4],
            )
            v2 = f2[:, 0:kc * 2].rearrange("p (t a) -> p t a", a=2)
            nc.vector.tensor_add(
                R[:, ks].rearrange("p (t a) -> p t a", a=1),
                v2[:, :, 0:1], v2[:, :, 1:2],
            )
            nc.scalar.mul(etaR[:, ks], R[:, ks], ETA)
            nc.scalar.activation(etaR2[:, ks], R[:, ks], AF.Square, scale=0.1)
            nc.scalar.square(R2[:, ks], R[:, ks])
            nc.vector.tensor_tensor_scan(
                out=Aloc[:, 1 + k0:1 + k0 + kc], data0=cvecL[:, 0:kc],
                data1=etaR[:, ks], initial=Aloc[:, k0:k0 + 1],
                op0=AL.mult, op1=AL.add,
            )
            nc.vector.tensor_tensor_scan(
                out=Bloc[:, 1 + k0:1 + k0 + kc], data0=cvecL[:, 0:kc],
                data1=etaR2[:, ks], initial=Bloc[:, k0:k0 + 1],
                op0=AL.mult, op1=AL.add,
            )

        # tail constant behind the bulk SWDGE stream
        nc.gpsimd.dma_start(geoc[:], geoc_dram.ap())

        # ---- tail ----
        # carries: previous partition's local final, via the SWDGE queue
        nc.gpsimd.dma_start(initA[1:KP, 0:1], Aloc[0:KP - 1, L:LE])
        nc.gpsimd.dma_start(initB[1:KP, 0:1], Bloc[0:KP - 1, L:LE])

        # A_prev[:,t] = Aloc[:,t-1] + initA*c^t  (Aloc col0 is the zero pad)
        nc.vector.scalar_tensor_tensor(
            out=Aprev[:, :], in0=geoc[:, :], scalar=initA[:, 0:1],
            in1=Aloc[:, 0:L], op0=AL.mult, op1=AL.add,
        )
        nc.vector.scalar_tensor_tensor(
            out=Bprev[:, :], in0=geoc[:, :], scalar=initB[:, 0:1],
            in1=Bloc[:, 0:L], op0=AL.mult, op1=AL.add,
        )

        # ---- D chain: q = [0.5*A*R^2 + B*(0.5*A - R)] / max(var,eps)^1.5 ----
        nc.vector.scalar_tensor_tensor(                             # h = 0.5A - R
            out=T1[:, :], in0=Aprev[:, :], scalar=0.5, in1=R[:, :],
            op0=AL.mult, op1=AL.subtract,
        )
        nc.vector.tensor_mul(geoc[:, :], Bprev[:, :], T1[:, :])     # B*h (geoc dead)
        nc.scalar.square(etaR2[:, :], Aprev[:, :])                  # a2 = A^2
        nc.vector.scalar_tensor_tensor(                             # g = 0.5A*R^2
            out=Aloc[:, 0:L], in0=Aprev[:, :], scalar=0.5, in1=R2[:, :],
            op0=AL.mult, op1=AL.mult,
        )
        nc.vector.tensor_sub(Bloc[:, 0:L], Bprev[:, :], etaR2[:, :])  # vraw = B-a2
        nc.scalar.sqrt(etaR[:, :], Bloc[:, 0:L])                    # s = sqrt(vraw)
        nc.vector.scalar_tensor_tensor(                             # d15=max(v,eps)*s
            out=R2[:, :], in0=Bloc[:, 0:L], scalar=EPS, in1=etaR[:, :],
            op0=AL.max, op1=AL.mult,
        )
        nc.vector.reciprocal_approx_accurate(etaR2[:, :], R2[:, :], Bprev[:, :])
        nc.vector.tensor_add(T1[:, :], Aloc[:, 0:L], geoc[:, :])    # negn
        nc.vector.scalar_tensor_tensor(                             # qsum=sum(negn*rec)
            out=Aprev[:, :], in0=T1[:, :], scalar=1.0, in1=etaR2[:, :],
            op0=AL.mult, op1=AL.mult, accum_out=qsum[:, 0:1],
        )
        # partition reduce: SWDGE gather to one row, DVE f32 reduce, store
        nc.gpsimd.dma_start(qrow[0:1, 0:KP - 1], qsum[1:KP, 0:1])
        nc.vector.reduce_sum(qtot[0:1, 0:1], qrow[0:1, 0:KP - 1], axis=AX.X)
        nc.sync.dma_start(out_ap[0:1, 0:1], qtot[0:1, 0:1])

    nc.compile()
    return nc


def _get_program():
    global _PROGRAM
    if _PROGRAM is None:
        _PROGRAM = _build_program()
    return _PROGRAM


def _fold16_f32(vals):
    """Exact f32 emulation of the device fold tree for one row's products."""
    v = [np.float32(x) for x in vals] + [np.float32(0)] * (16 - len(vals))
    t8 = [np.float32(np.float32(v[i]) + np.float32(v[i + 8])) for i in range(8)]
    t4 = [np.float32(t8[i] + t8[i + 4]) for i in range(4)]
    t2 = [np.float32(t4[i] + t4[i + 2]) for i in range(2)]
    return np.float32(t2[0] + t2[1])


def _core0_prepend():
    """2000 synthetic rows encoding the global init (A,B)=(~0,~eps).

    Rows 0..1997 are zero. Row 1998 returns R_x (base bf16 value + trim
    asset u); row 1999 returns -(t1+t2+t3), a 3-term bf16 decomposition
    of c*R_x, so the f32 A-scan cancels to ~1e-12 while the B-scan lands
    on eps. u is chosen by exact f32 simulation of the device pipeline.
    """
    ETA32, C32, P1 = NF32(ETA), NF32(CDEC), NF32(0.1)
    EPS32 = NF32(EPS)
    c = float(CDEC)
    r1 = float(np.sqrt(EPS / (ETA * (c + c ** 2))))
    s_cands = {float(NBF16(r1 * (1.0 + d * 2e-3))) for d in range(-2, 3)}
    u_cands = {float(NBF16(x)) for x in np.linspace(-4e-6, 4e-6, 2400)}
    best = None
    for s in s_cands:
        for u in u_cands:
            R_x = _fold16_f32([s, u])
            target = -c * float(R_x)
            t1 = NBF16(target)
            t2 = NBF16(target - float(t1))
            t3 = NBF16(target - float(t1) - float(t2))
            R_x1 = _fold16_f32([float(t1), float(t2), float(t3)])
            a1 = NF32(ETA32 * R_x)
            a2 = NF32(NF32(C32 * a1) + NF32(ETA32 * R_x1))
            b1 = NF32(NF32(P1 * R_x) * NF32(P1 * R_x))
            b2 = NF32(NF32(C32 * b1) + NF32(NF32(P1 * R_x1) * NF32(P1 * R_x1)))
            score = abs(float(b2) - float(EPS32)) * 1.1e10 + abs(float(a2)) * 6.3e9
            if best is None or score < best[0]:
                best = (score, s, u, t1, t2, t3)
    _, s, u, t1, t2, t3 = best
    w = np.zeros((L, NA), NBF16)
    nr = np.zeros((L, NA), NBF16)
    w[L - 2, 0:2] = NBF16(1.0)
    nr[L - 2, 0] = NBF16(s)
    nr[L - 2, 1] = NBF16(u)
    w[L - 1, 0:3] = NBF16(1.0)
    nr[L - 1, 0] = t1
    nr[L - 1, 1] = t2
    nr[L - 1, 2] = t3
    return w, nr


def _make_in_maps(weights, nr):
    wb = np.asarray(weights, dtype=NF32).astype(NBF16)
    nb = np.asarray(nr, dtype=NF32).astype(NBF16)
    pre_w, pre_nr = _core0_prepend()
    in_maps = []
    for m in range(N_CORES):
        s = m * OWN
        if m == 0:
            wm = np.concatenate([pre_w, wb[:OWN]])
            rm = np.concatenate([pre_nr, nb[:OWN]])
        else:
            wm = wb[s - L:s + OWN]
            rm = nb[s - L:s + OWN]
        in_maps.append({"w": np.ascontiguousarray(wm), "nr": np.ascontiguousarray(rm)})
    return in_maps


def _run(in_maps, **kwargs):
    nc = _get_program()
    return run_bass_kernel_spmd(nc, in_maps, core_ids=list(range(N_CORES)), **kwargs)


def kernel(weights, next_returns):
    in_maps = _make_in_maps(weights, next_returns)
    res = _run(in_maps)
    total = np.sum(
        np.array([res.results[m]["out"][0, 0] for m in range(N_CORES)], NF32),
        dtype=NF32,
    )
    return NF32(NF32(ETA) * total / NF32(B_TOTAL))
